# revision 18
# baseline (speedup 1.0000x reference)
"""Selective SSM (Mamba-1 style) layer on 8 Trainium2 NeuronCores — v5.

Sharding: core c -> batch b = c // 2, d_model half dh = c % 2 (512 channels).
Cores fully independent (recurrence elementwise in d); no collectives.

The DVE tensor_tensor_scan is the hard bottleneck: 4.42 us per [128,2048]
tile regardless of dtype (2 cyc/elem, no fast modes), 64 tiles = 283 us.
DVE does scans + u/prod muls (2x mode, ~1.1 us each); all other work hides
under it. v5 pipeline fixes vs v4 (534 us):
  - per-core channel permutation: each core's own 512 channels are columns
    0..511 of its x/weight copies, so x^T tiles 0..3 double as the scan
    slice (drops the separate xsl16 load: -16KB SBUF, -4 xbar DMAs).
  - delta projection interleaved into the first scan m-loop (v4 left the
    DVE idle 53 us waiting on all projections).
  - all 32 B/C broadcasts issued upfront in their own 16-slot ring,
    B-rows on the sync DMA queue, C-rows on the scalar-engine DMA queue
    (v4 stalled ~42 us at the n-half boundary on broadcast DMAs).
  - y output (xbar transpose + store) per-m right after its last plane.
  - dskx and y16 share one tile per m (dskx dead once the half-0 opener
    planes consumed it).
Structure retained from v4:
  - n-reduction via PE identity-matmul planes; skip term opens half 0,
    the running y16 opens half 1; ACT copies PSUM -> y16 bf16.
  - bar exps on ACT (bf16), softplus Exp in-place on PSUM.
  - y bf16, transposed by SBUF->SBUF DMA xbar, upcast in numpy.
"""

import numpy as np
import ml_dtypes
from contextlib import ExitStack

import concourse.bacc as bacc
import concourse.bass as bass
import concourse.mybir as mybir
import concourse.tile as tile
from concourse.bass_utils import run_bass_kernel_spmd

BF16 = ml_dtypes.bfloat16
F32 = mybir.dt.float32
B16 = mybir.dt.bfloat16

B_SZ, SEQ, D, N = 4, 2048, 1024, 16
DL = 512            # d_model channels per core
ND = DL // 128      # 4 d-tiles
NK = D // 128       # 8 contraction tiles
TB = SEQ // 512     # 4 moving-dim blocks for matmul
NHALF = 2
NH = N // NHALF     # 8 states per half

_CACHE = {}


def _build():
    if "nc" in _CACHE:
        return _CACHE["nc"]
    mult = mybir.AluOpType.mult
    add = mybir.AluOpType.add

    nc = bacc.Bacc("TRN2", target_bir_lowering=False, debug=False, num_devices=8)

    # x arrives pre-transposed [D, SEQ] and channel-permuted (own 512 first).
    xb16_d = nc.dram_tensor("xb16", [D, SEQ], B16, kind="ExternalInput")
    wd16_d = nc.dram_tensor("wd16", [D, DL], B16, kind="ExternalInput")
    wb16_d = nc.dram_tensor("wb16", [D, N], B16, kind="ExternalInput")
    wc16_d = nc.dram_tensor("wc16", [D, N], B16, kind="ExternalInput")
    aneg_d = nc.dram_tensor("aneg", [DL, N], F32, kind="ExternalInput")
    bdsk_d = nc.dram_tensor("bdsk", [DL, 2], F32, kind="ExternalInput")
    bbc_d = nc.dram_tensor("bbc", [N, 2], F32, kind="ExternalInput")
    id16_d = nc.dram_tensor("id16", [128, 128], B16, kind="ExternalInput")
    y_d = nc.dram_tensor("y", [SEQ, DL], B16, kind="ExternalOutput")

    with tile.TileContext(nc) as tc, ExitStack() as ctx:
        consts = ctx.enter_context(tc.tile_pool(name="consts", bufs=1))
        persist = ctx.enter_context(tc.tile_pool(name="persist", bufs=1))
        ps_mm = ctx.enter_context(tc.tile_pool(name="ps_mm", bufs=2, space="PSUM"))
        ps_y = ctx.enter_context(tc.tile_pool(name="ps_y", bufs=1, space="PSUM"))
        xpool = ctx.enter_context(tc.tile_pool(name="xpool", bufs=8))
        bcast = ctx.enter_context(tc.tile_pool(name="bcast", bufs=19))
        work = ctx.enter_context(tc.tile_pool(name="work", bufs=2))
        dram = ctx.enter_context(tc.tile_pool(name="dram", bufs=1, space="DRAM"))

        # ---- weights + x^T, interleaved across both DMA queues ----
        wdall = consts.tile([128, NK * DL], B16, tag="wdall", name="wdall")
        wball = consts.tile([128, NK * N], B16, tag="wball", name="wball")
        wcall = consts.tile([128, NK * N], B16, tag="wcall", name="wcall")
        # small B/C weights first (the B/C projection is the critical path)
        for k in range(NK):
            nc.sync.dma_start(wball[:, k * N:(k + 1) * N],
                              wb16_d[k * 128:(k + 1) * 128, :])
            nc.scalar.dma_start(wcall[:, k * N:(k + 1) * N],
                                wc16_d[k * 128:(k + 1) * 128, :])
        xt = []
        for k in range(NK):
            t = xpool.tile([128, SEQ], B16, tag="xt", name=f"xt{k}")
            eng = nc.sync if k % 2 == 0 else nc.scalar
            eng.dma_start(t[:], xb16_d[k * 128:(k + 1) * 128, :])
            xt.append(t)
        for k in range(NK):
            nc.scalar.dma_start(wdall[:, k * DL:(k + 1) * DL],
                                wd16_d[k * 128:(k + 1) * 128, :])
        wd_sb = [wdall[:, k * DL:(k + 1) * DL] for k in range(NK)]
        wb_sb = [wball[:, k * N:(k + 1) * N] for k in range(NK)]
        wc_sb = [wcall[:, k * N:(k + 1) * N] for k in range(NK)]
        abd = []
        for m in range(ND):
            t = consts.tile([128, N + 2], F32, tag=f"abd{m}", name=f"abd{m}")
            nc.sync.dma_start(t[:, 0:N], aneg_d[m * 128:(m + 1) * 128, :])
            nc.sync.dma_start(t[:, N:N + 2], bdsk_d[m * 128:(m + 1) * 128, :])
            abd.append(t)
        a_sb = [t[:, 0:N] for t in abd]
        bd_sb = [t[:, N:N + 1] for t in abd]
        dsk_sb = [t[:, N + 1:N + 2] for t in abd]
        bbc = consts.tile([N, 2], F32, tag="bbc", name="bbc")
        nc.sync.dma_start(bbc[:], bbc_d[:, :])
        bb_sb = bbc[:, 0:1]
        bc_sb = bbc[:, 1:2]
        id16_sb = consts.tile([128, 128], B16, tag="id16", name="id16sb")
        nc.sync.dma_start(id16_sb[:], id16_d[:, :])

        # ---- B/C projections -> bmat/cmat [N, SEQ] bf16 ----
        bmat = bcast.tile([128, SEQ], B16, tag="bc16", name="bmat")[0:N, :]
        cmat = bcast.tile([128, SEQ], B16, tag="bc16", name="cmat")[0:N, :]
        for tb in range(TB):
            psb = ps_mm.tile([128, 1024], F32, tag="mm", name="mmpb")
            for k in range(NK):
                nc.tensor.matmul(
                    psb[0:N, 0:512], wb_sb[k], xt[k][:, tb * 512:(tb + 1) * 512],
                    start=(k == 0), stop=(k == NK - 1),
                )
            for k in range(NK):
                nc.tensor.matmul(
                    psb[0:N, 512:1024], wc_sb[k],
                    xt[k][:, tb * 512:(tb + 1) * 512],
                    start=(k == 0), stop=(k == NK - 1),
                )
            nc.scalar.activation(
                bmat[:, tb * 512:(tb + 1) * 512], psb[0:N, 0:512],
                mybir.ActivationFunctionType.Identity, bias=bb_sb, scale=1.0,
            )
            nc.scalar.activation(
                cmat[:, tb * 512:(tb + 1) * 512], psb[0:N, 512:1024],
                mybir.ActivationFunctionType.Identity, bias=bc_sb, scale=1.0,
            )

        # bounce B/C to DRAM, then issue ALL 32 broadcasts upfront.
        # B rows ride the sync DMA queue, C rows the scalar-engine queue.
        bmat_dr = dram.tile([N, SEQ], B16, tag="bmat_dr", name="bmat_dr")
        cmat_dr = dram.tile([N, SEQ], B16, tag="cmat_dr", name="cmat_dr")
        for tb in range(TB):
            ts = slice(tb * 512, (tb + 1) * 512)
            nc.sync.dma_start(bmat_dr[:, ts], bmat[:, ts])
            nc.scalar.dma_start(cmat_dr[:, ts], cmat[:, ts])
        breps = []
        creps = []
        for n in range(N):
            br = bcast.tile([128, SEQ], B16, tag="bc16", name="brep")
            cr = bcast.tile([128, SEQ], B16, tag="bc16", name="crep")
            for tb in range(TB):
                ts = slice(tb * 512, (tb + 1) * 512)
                nc.sync.dma_start(
                    br[:, ts], bmat_dr[n:n + 1, ts].partition_broadcast(128))
                nc.scalar.dma_start(
                    cr[:, ts], cmat_dr[n:n + 1, ts].partition_broadcast(128))
            breps.append(br)
            creps.append(cr)

        # ---- persist tiles for the scan ----
        dt16 = [persist.tile([128, SEQ], B16, tag=f"dt{m}", name=f"dtv{m}")
                for m in range(ND)]
        dtx = [persist.tile([128, SEQ], B16, tag=f"dtx{m}", name=f"dtx{m}")
               for m in range(ND)]
        # yd16[m]: holds dskx until the half-0 opener consumed it, then the
        # running/final y16.
        yd16 = [persist.tile([128, SEQ], B16, tag=f"yd{m}", name=f"yd{m}")
                for m in range(ND)]

        def delta_proj(m):
            pss = []
            for th in range(2):
                ps = ps_mm.tile([128, 1024], F32, tag="mm", name="mmps")
                for sb in range(2):
                    for k in range(NK):
                        nc.tensor.matmul(
                            ps[:, sb * 512:(sb + 1) * 512],
                            wd_sb[k][:, m * 128:(m + 1) * 128],
                            xt[k][:, th * 1024 + sb * 512:th * 1024 + (sb + 1) * 512],
                            start=(k == 0), stop=(k == NK - 1),
                        )
                nc.scalar.activation(
                    ps[:], ps[:], mybir.ActivationFunctionType.Exp,
                    bias=bd_sb[m], scale=1.0,
                )
                pss.append(ps)
            for th in range(2):
                nc.scalar.activation(
                    dt16[m][:, th * 1024:(th + 1) * 1024], pss[th][:],
                    mybir.ActivationFunctionType.Ln, bias=1.0, scale=1.0,
                )
            nc.vector.tensor_mul(dtx[m][:], dt16[m][:], xt[m][:])
            nc.vector.tensor_scalar_mul(yd16[m][:], xt[m][:], dsk_sb[m])

        for m in range(ND):
            delta_proj(m)

        # ---- scan phase ----
        for half in range(NHALF):
            for m in range(ND):
                yps = ps_y.tile([128, SEQ], F32, tag="yps", name="ypsv")
                for tb in range(TB):
                    nc.tensor.matmul(
                        yps[:, tb * 512:(tb + 1) * 512], id16_sb[:],
                        yd16[m][:, tb * 512:(tb + 1) * 512],
                        start=True, stop=False,
                    )
                for j in range(NH):
                    n = half * NH + j
                    bar = work.tile([128, SEQ], B16, tag="bar", name="barv")
                    nc.scalar.activation(
                        bar[:], dt16[m][:],
                        mybir.ActivationFunctionType.Exp,
                        bias=0.0, scale=a_sb[m][:, n:n + 1],
                    )
                    u = work.tile([128, SEQ], B16, tag="u", name="uv")
                    nc.vector.tensor_mul(u[:], dtx[m][:], breps[n][:])
                    h = work.tile([128, SEQ], B16, tag="h", name="hv")
                    nc.vector.tensor_tensor_scan(
                        h[:], bar[:], u[:], 0.0, op0=mult, op1=add,
                    )
                    prod = work.tile([128, SEQ], B16, tag="prod", name="prodv")
                    nc.vector.tensor_mul(prod[:], h[:], creps[n][:])
                    last = (j == NH - 1)
                    for tb in range(TB):
                        nc.tensor.matmul(
                            yps[:, tb * 512:(tb + 1) * 512], id16_sb[:],
                            prod[:, tb * 512:(tb + 1) * 512],
                            start=False, stop=last,
                        )
                if half == 0:
                    nc.scalar.activation(
                        yd16[m][:], yps[:],
                        mybir.ActivationFunctionType.Copy, bias=0.0, scale=1.0,
                    )
                else:
                    for sh in range(2):
                        hs = slice(sh * (SEQ // 2), (sh + 1) * (SEQ // 2))
                        nc.scalar.activation(
                            yd16[m][:, hs], yps[:, hs],
                            mybir.ActivationFunctionType.Copy,
                            bias=0.0, scale=1.0,
                        )
                        ytt = work.tile([128, SEQ // 256, 128], B16, tag="ytt",
                                        name="yttv")
                        nc.sync.dma_start_transpose(ytt[:], yd16[m][:, hs])
                        nc.sync.dma_start(
                            y_d[sh * (SEQ // 2):(sh + 1) * (SEQ // 2),
                                m * 128:(m + 1) * 128]
                            .rearrange("(j p) q -> p j q", p=128),
                            ytt[:],
                        )

    nc.compile()
    _CACHE["nc"] = nc
    return nc


def _in_maps(x, A_log, D_skip, Wd, bd, Wb, bb, Wc, bc):
    A = (-np.exp(np.asarray(A_log, np.float64))).astype(np.float32)
    x = np.asarray(x, np.float32)
    maps = []
    for c in range(8):
        b, dh = c // 2, c % 2
        dsl = slice(dh * DL, (dh + 1) * DL)
        osl = slice((1 - dh) * DL, (2 - dh) * DL)
        perm = np.r_[np.arange(dsl.start, dsl.stop),
                     np.arange(osl.start, osl.stop)]
        bdsk = np.stack([np.asarray(bd, np.float32)[dsl],
                         np.asarray(D_skip, np.float32)[dsl]], axis=1)
        bbcm = np.stack([np.asarray(bb, np.float32),
                         np.asarray(bc, np.float32)], axis=1)
        maps.append({
            "xb16": np.ascontiguousarray(x[b][:, perm].astype(BF16).T),
            "wd16": np.asarray(Wd)[perm][:, dsl].astype(BF16),
            "wb16": np.asarray(Wb)[perm].astype(BF16),
            "wc16": np.asarray(Wc)[perm].astype(BF16),
            "aneg": A[dsl],
            "bdsk": np.ascontiguousarray(bdsk),
            "bbc": np.ascontiguousarray(bbcm),
            "id16": np.eye(128, dtype=BF16),
        })
    return maps


def kernel(x, A_log, D_skip, Wd, bd, Wb, bb, Wc, bc, _trace=False):
    nc = _build()
    maps = _in_maps(x, A_log, D_skip, Wd, bd, Wb, bb, Wc, bc)
    res = run_bass_kernel_spmd(nc, maps, list(range(8)), trace=_trace)
    y = np.zeros((B_SZ, SEQ, D), np.float32)
    for c, om in enumerate(res.results):
        b, dh = c // 2, c % 2
        y[b][:, dh * DL:(dh + 1) * DL] = om["y"].astype(np.float32)
    if _trace:
        kernel.last_result = res
    return y
